# revision 7
# baseline (speedup 1.0000x reference)
"""Trainium2 Bass kernel: single-head causal attention, SPMD over 8 NeuronCores.

Problem: x [4, 2048, 1024] f32; Wq/Wk/Wv [1024, 64]; bq/bk/bv [64].
  q,k,v = x@W + b ; out = softmax(causal(q k^T / 8)) @ v  -> [4, 2048, 64]

Sharding (uniform SPMD structure on every core):
  core c -> batch b = c//2 ; query chunks (cA, cB) = (c%2, 3-c%2), 512 rows each.
  Every core computes K/V for its batch's full 2048 rows (cheaper than
  exchanging them via collectives at this size), Q for its own 1024 rows.
  Chunk slot A structurally uses K-tiles 0..7 (its kv extent <= 1024 always),
  slot B uses K-tiles 0..15; causality inside tiles is enforced by a
  data-driven (q_iota >= thr) mask, so one graph serves all cores.

Layouts (all host-prepared so every DMA is a contiguous [part, free] copy):
  projections produce Q^T/K^T/V^T [64, rows]; scores are computed transposed
  ([k_part, q_free]) so the attention-weight matrix feeds the AV matmul
  directly as the moving operand; V is re-transposed in 16 small PE
  transposes; a 65th "ones" row on the V tiles makes the AV matmul also
  accumulate the softmax denominator.

dtypes: fp16 SBUF operands (1 cycle/row on the PE, integers exact to 2048 for
  the iota/thr mask), fp32 PSUM accumulation, fp32 epilogue + output.
"""

import os
import sys

import numpy as np

if "/opt/trn_rl_repo" not in sys.path:
    sys.path.insert(0, "/opt/trn_rl_repo")

B, S, D, H = 4, 2048, 1024, 64
CH = 512          # query chunk width
QR = 2 * CH       # query rows per core
NKT = S // 128    # 16 k-tiles of 128
SLOT_KT = (8, 16)  # k-tiles consumed by slot A / slot B
SCALE = 1.0 / np.sqrt(H)

_CACHE = {}


def _build_nc():
    import concourse.bacc as bacc
    import concourse.mybir as mybir
    import concourse.tile as tile

    DT = mybir.dt.float16
    F32 = mybir.dt.float32
    Exp = mybir.ActivationFunctionType.Exp
    ge = mybir.AluOpType.is_ge
    mult = mybir.AluOpType.mult
    add = mybir.AluOpType.add

    nc = bacc.Bacc("TRN2", target_bir_lowering=False, debug=False, num_devices=8)

    xkT = nc.dram_tensor("xkT", [D, S], DT, kind="ExternalInput")
    xqT = nc.dram_tensor("xqT", [D, QR], DT, kind="ExternalInput")
    wkv = nc.dram_tensor("wkv", [128, 8 * 128], DT, kind="ExternalInput")
    wq = nc.dram_tensor("wq", [128, 8 * H], DT, kind="ExternalInput")
    bkv = nc.dram_tensor("bkv", [128, 1], F32, kind="ExternalInput")
    bq = nc.dram_tensor("bq", [H, 1], F32, kind="ExternalInput")
    qio = nc.dram_tensor("qio", [128, CH], DT, kind="ExternalInput")
    thr = nc.dram_tensor("thr", [128, 2 * NKT], DT, kind="ExternalInput")
    idv = nc.dram_tensor("idv", [128, H], DT, kind="ExternalInput")
    id32 = nc.dram_tensor("id32", [128, 128], F32, kind="ExternalInput")
    out = nc.dram_tensor("out", [QR, H], F32, kind="ExternalOutput")

    with tile.TileContext(nc) as tc:
        with (
            tc.tile_pool(name="const", bufs=1) as cp,
            tc.tile_pool(name="work", bufs=3) as wp,
            tc.tile_pool(name="epi", bufs=2) as ep,
        ):
            # ---- constants / inputs to SBUF ----
            xk_sb = []
            for kt in range(8):
                t = cp.tile([128, S], DT, tag=f"xk{kt}", name=f"xk{kt}")
                nc.sync.dma_start(t[:], xkT[kt * 128:(kt + 1) * 128, :])
                xk_sb.append(t)
            xq_sb = []
            for kt in range(8):
                t = cp.tile([128, QR], DT, tag=f"xq{kt}", name=f"xq{kt}")
                nc.sync.dma_start(t[:], xqT[kt * 128:(kt + 1) * 128, :])
                xq_sb.append(t)
            wkv_sb = cp.tile([128, 8 * 128], DT, tag="wkv", name="wkv")
            nc.sync.dma_start(wkv_sb[:], wkv[:])
            wq_sb = cp.tile([128, 8 * H], DT, tag="wq", name="wq")
            nc.sync.dma_start(wq_sb[:], wq[:])
            bkv_sb = cp.tile([128, 1], F32, tag="bkv", name="bkv")
            nc.sync.dma_start(bkv_sb[:], bkv[:])
            bq_sb = cp.tile([H, 1], F32, tag="bq", name="bq")
            nc.sync.dma_start(bq_sb[:], bq[:])
            qio_sb = cp.tile([128, CH], DT, tag="qio", name="qio")
            nc.sync.dma_start(qio_sb[:], qio[:])
            thr_sb = cp.tile([128, 2 * NKT], DT, tag="thr", name="thr")
            nc.sync.dma_start(thr_sb[:], thr[:])
            idv_sb = cp.tile([128, H], DT, tag="idv", name="idv")
            nc.sync.dma_start(idv_sb[:], idv[:])
            id32_sb = cp.tile([128, 128], F32, tag="id32", name="id32")
            nc.sync.dma_start(id32_sb[:], id32[:])

            kvT_sb = cp.tile([128, S], DT, tag="kvT", name="kvT")     # rows 0:64 K^T, 64:128 V^T
            qT_sb = cp.tile([H, QR], DT, tag="qT", name="qT")
            v_sb = cp.tile([128, NKT * (H + 1)], DT, tag="v", name="v")  # V tiles + ones col

            # ---- phase 1: projections (own PSUM scope: 6 banks, freed after) ----
            with tc.tile_pool(name="proj_ps", bufs=1, space="PSUM") as pp:
                kv_ps = [pp.tile([128, 512], F32, tag=f"kvps{nb}", name=f"kvps{nb}") for nb in range(4)]
                q_ps = [pp.tile([H, 512], F32, tag=f"qps{j}", name=f"qps{j}") for j in range(2)]
                for kt in range(8):
                    lw = wkv_sb[:, kt * 128:(kt + 1) * 128]
                    lq = wq_sb[:, kt * H:(kt + 1) * H]
                    for nb in range(4):
                        nc.tensor.matmul(
                            kv_ps[nb][:],
                            lw,
                            xk_sb[kt][:, nb * 512:(nb + 1) * 512],
                            start=(kt == 0),
                            stop=(kt == 7),
                        )
                    for j in range(2):
                        nc.tensor.matmul(
                            q_ps[j][:],
                            lq,
                            xq_sb[kt][:, j * 512:(j + 1) * 512],
                            start=(kt == 0),
                            stop=(kt == 7),
                        )
                for nb in range(4):
                    nc.vector.tensor_scalar(
                        kvT_sb[:, nb * 512:(nb + 1) * 512], kv_ps[nb][:],
                        bkv_sb[:], None, add)
                for j in range(2):
                    nc.vector.tensor_scalar(
                        qT_sb[:, j * 512:(j + 1) * 512], q_ps[j][:],
                        bq_sb[:], None, add)

            # ---- phase 1c + 2 PSUM pools: vtr 2 + score 2 + av 2 + otr 2 = 8 banks
            vp = tc.alloc_tile_pool(name="vtr_ps", bufs=2, space="PSUM")
            sp = tc.alloc_tile_pool(name="score_ps", bufs=2, space="PSUM")
            avp = tc.alloc_tile_pool(name="av_ps", bufs=1, space="PSUM")
            op = tc.alloc_tile_pool(name="otr_ps", bufs=2, space="PSUM")

            # ---- phase 1c: V^T -> V tiles (+ ones column) ----
            nc.vector.memset(v_sb[:], 1.0)
            for kt in range(NKT):
                vt_ps = vp.tile([128, H], DT, tag="vtr", name="vtr")
                nc.tensor.transpose(
                    vt_ps[:],
                    kvT_sb[64:128, kt * 128:(kt + 1) * 128],
                    idv_sb[64:64 + H, :],
                )
                nc.vector.tensor_copy(
                    v_sb[:, kt * (H + 1):kt * (H + 1) + H], vt_ps[:])

            # ---- phase 2: attention per slot ----
            for slot in range(2):
                nkt = SLOT_KT[slot]
                av = avp.tile([H + 1, 512], F32, tag=f"av{slot}", name=f"av{slot}")
                for kt in range(nkt):
                    s_ps = sp.tile([128, 512], F32, tag="score", name="score")
                    nc.tensor.matmul(
                        s_ps[:],
                        kvT_sb[0:H, kt * 128:(kt + 1) * 128],
                        qT_sb[:, slot * 512:(slot + 1) * 512],
                        start=True, stop=True,
                    )
                    w_sb = wp.tile([128, 512], DT, tag="wexp", name="wexp")
                    nc.scalar.activation(w_sb[:], s_ps[:], Exp, scale=float(SCALE))
                    wm_sb = wp.tile([128, 512], DT, tag="wm", name="wm")
                    idx = slot * NKT + kt
                    nc.vector.scalar_tensor_tensor(
                        wm_sb[:], qio_sb[:], thr_sb[:, idx:idx + 1], w_sb[:],
                        ge, mult)
                    nc.tensor.matmul(
                        av[:],
                        v_sb[:, kt * (H + 1):(kt + 1) * (H + 1)],
                        wm_sb[:],
                        start=(kt == 0), stop=(kt == nkt - 1),
                    )
                # epilogue: transpose [65, 512] -> 4x [128, 65], normalize, store
                oav_sb = ep.tile([H + 1, 512], F32, tag="oav", name="oav")
                nc.vector.tensor_copy(oav_sb[:], av[:])
                for j in range(4):
                    tr_ps = op.tile([128, H + 1], F32, tag="otr", name="otr")
                    nc.tensor.transpose(
                        tr_ps[:],
                        oav_sb[:, j * 128:(j + 1) * 128],
                        id32_sb[0:H + 1, 0:H + 1],
                    )
                    r_sb = ep.tile([128, 1], F32, tag="recip", name="recip")
                    nc.vector.reciprocal(r_sb[:], tr_ps[:, H:H + 1])
                    o_sb = ep.tile([128, H], F32, tag="osb", name="osb")
                    nc.vector.tensor_scalar_mul(o_sb[:], tr_ps[:, 0:H], r_sb[:])
                    row = slot * CH + j * 128
                    nc.sync.dma_start(out[row:row + 128, :], o_sb[:])

            for pool in (op, avp, sp, vp):
                pool.release()

    nc.compile()
    return nc


def _host_inputs(x, Wq, bq, Wk, bk, Wv, bv):
    """Build the 8 per-core input maps (all SBUF-layout, fp16/f32)."""
    f16 = np.float16
    Wkv = np.concatenate([Wk, Wv], axis=1)          # [D, 128]
    wkv_np = np.zeros((128, 8 * 128), dtype=f16)
    wq_np = np.zeros((128, 8 * H), dtype=f16)
    for kt in range(8):
        wkv_np[:, kt * 128:(kt + 1) * 128] = Wkv[kt * 128:(kt + 1) * 128, :]
        wq_np[:, kt * H:(kt + 1) * H] = Wq[kt * 128:(kt + 1) * 128, :]
    bkv_np = np.concatenate([bk, bv]).reshape(128, 1).astype(np.float32)
    bq_np = bq.reshape(H, 1).astype(np.float32)
    qio_np = np.broadcast_to(
        np.arange(CH, dtype=f16), (128, CH)).copy()
    idv_np = np.concatenate([np.eye(H), np.eye(H)], axis=0).astype(f16)
    id32_np = np.eye(128, dtype=np.float32)

    in_maps = []
    for c in range(8):
        b = c // 2
        cA, cB = c % 2, 3 - c % 2
        xkT_np = np.ascontiguousarray(x[b].T).astype(f16)        # [D, S]
        xqT_np = np.concatenate(
            [x[b, cA * CH:(cA + 1) * CH].T, x[b, cB * CH:(cB + 1) * CH].T],
            axis=1).astype(f16)                                   # [D, QR]
        thr_np = np.zeros((128, 2 * NKT), dtype=f16)
        p = np.arange(128)
        for slot, ck in enumerate((cA, cB)):
            for kt in range(NKT):
                thr_np[:, slot * NKT + kt] = kt * 128 + p - ck * CH
        in_maps.append({
            "xkT": xkT_np, "xqT": xqT_np, "wkv": wkv_np, "wq": wq_np,
            "bkv": bkv_np, "bq": bq_np, "qio": qio_np, "thr": thr_np,
            "idv": idv_np, "id32": id32_np,
        })
    return in_maps


def _gather(results, dtype):
    y = np.zeros((B, S, H), dtype=dtype)
    for c in range(8):
        b = c // 2
        cA, cB = c % 2, 3 - c % 2
        o = results[c]["out"]
        y[b, cA * CH:(cA + 1) * CH] = o[:CH]
        y[b, cB * CH:(cB + 1) * CH] = o[CH:]
    return y


def get_nc():
    if "nc" not in _CACHE:
        _CACHE["nc"] = _build_nc()
    return _CACHE["nc"]


def kernel(x, Wq, bq, Wk, bk, Wv, bv, _trace=False, _trace_kwargs=None):
    from concourse.bass_utils import run_bass_kernel_spmd

    x = np.asarray(x, dtype=np.float32)
    Wq, bq = np.asarray(Wq, np.float32), np.asarray(bq, np.float32)
    Wk, bk = np.asarray(Wk, np.float32), np.asarray(bk, np.float32)
    Wv, bv = np.asarray(Wv, np.float32), np.asarray(bv, np.float32)

    nc = get_nc()
    in_maps = _host_inputs(x, Wq, bq, Wk, bk, Wv, bv)
    res = run_bass_kernel_spmd(
        nc, in_maps, core_ids=list(range(8)),
        trace=_trace, **(_trace_kwargs or {}))
    _CACHE["last_result"] = res
    return _gather(res.results, x.dtype)


# revision 9
# speedup vs baseline: 1.1152x; 1.1152x over previous
"""Trainium2 Bass kernel: single-head causal attention, SPMD over 8 NeuronCores.

Problem: x [4, 2048, 1024] f32; Wq/Wk/Wv [1024, 64]; bq/bk/bv [64].
  q,k,v = x@W + b ; out = softmax(causal(q k^T / 8)) @ v  -> [4, 2048, 64]

Sharding (uniform SPMD structure on every core):
  core c -> batch b = c//2 ; query chunks (cA, cB) = (c%2, 3-c%2), 512 rows each.
  Every core computes K/V for its batch's full 2048 rows (cheaper than
  exchanging them via collectives at this size), Q for its own 1024 rows.
  Chunk slot A structurally uses K-tiles 0..7 (its kv extent <= 1024 always),
  slot B uses K-tiles 0..15; causality inside tiles is enforced by a
  data-driven (q_iota >= thr) mask, so one graph serves all cores.

Layouts (all host-prepared so every DMA is a contiguous [part, free] copy):
  projections produce Q^T/K^T/V^T [64, rows]; scores are computed transposed
  ([k_part, q_free]) so the attention-weight matrix feeds the AV matmul
  directly as the moving operand; V is re-transposed in 16 small PE
  transposes; a 65th "ones" row on the V tiles makes the AV matmul also
  accumulate the softmax denominator.

dtypes: fp16 SBUF operands (1 cycle/row on the PE, integers exact to 2048 for
  the iota/thr mask), fp32 PSUM accumulation, fp32 epilogue + output.
"""

import os
import sys

import numpy as np

if "/opt/trn_rl_repo" not in sys.path:
    sys.path.insert(0, "/opt/trn_rl_repo")

B, S, D, H = 4, 2048, 1024, 64
CH = 512          # query chunk width
QR = 2 * CH       # query rows per core
NKT = S // 128    # 16 k-tiles of 128
SLOT_KT = (8, 16)  # k-tiles consumed by slot A / slot B
SCALE = 1.0 / np.sqrt(H)

_CACHE = {}


def _build_nc():
    import concourse.bacc as bacc
    import concourse.mybir as mybir
    import concourse.tile as tile

    DT = mybir.dt.float16
    F32 = mybir.dt.float32
    Exp = mybir.ActivationFunctionType.Exp
    ge = mybir.AluOpType.is_ge
    mult = mybir.AluOpType.mult
    add = mybir.AluOpType.add

    nc = bacc.Bacc("TRN2", target_bir_lowering=False, debug=False, num_devices=8)

    xkT = nc.dram_tensor("xkT", [D, S], DT, kind="ExternalInput")
    xqT = nc.dram_tensor("xqT", [D, QR], DT, kind="ExternalInput")
    wkv = nc.dram_tensor("wkv", [128, 8 * 128], DT, kind="ExternalInput")
    wq = nc.dram_tensor("wq", [128, 8 * H], DT, kind="ExternalInput")
    bkv = nc.dram_tensor("bkv", [128, 1], F32, kind="ExternalInput")
    bq = nc.dram_tensor("bq", [H, 1], F32, kind="ExternalInput")
    qio = nc.dram_tensor("qio", [128, CH], DT, kind="ExternalInput")
    thr = nc.dram_tensor("thr", [128, 2 * NKT], DT, kind="ExternalInput")
    idv = nc.dram_tensor("idv", [128, H], DT, kind="ExternalInput")
    id32 = nc.dram_tensor("id32", [128, 128], F32, kind="ExternalInput")
    out = nc.dram_tensor("out", [QR, H], F32, kind="ExternalOutput")

    with tile.TileContext(nc) as tc:
        with (
            tc.tile_pool(name="const", bufs=1) as cp,
            tc.tile_pool(name="work", bufs=4) as wp,
            tc.tile_pool(name="epi", bufs=2) as ep,
        ):
            # ---- constants / inputs to SBUF (small first: they gate the PE) --
            wkv_sb = cp.tile([128, 8 * 128], DT, tag="wkv", name="wkv")
            nc.sync.dma_start(wkv_sb[:], wkv[:])
            wq_sb = cp.tile([128, 8 * H], DT, tag="wq", name="wq")
            nc.sync.dma_start(wq_sb[:], wq[:])
            bkv_sb = cp.tile([128, 1], F32, tag="bkv", name="bkv")
            nc.gpsimd.dma_start(bkv_sb[:], bkv[:])
            bq_sb = cp.tile([H, 1], F32, tag="bq", name="bq")
            nc.gpsimd.dma_start(bq_sb[:], bq[:])
            qio_sb = cp.tile([128, CH], DT, tag="qio", name="qio")
            nc.gpsimd.dma_start(qio_sb[:], qio[:])
            thr_sb = cp.tile([128, 2 * NKT], DT, tag="thr", name="thr")
            nc.gpsimd.dma_start(thr_sb[:], thr[:])
            idv_sb = cp.tile([128, H], DT, tag="idv", name="idv")
            nc.gpsimd.dma_start(idv_sb[:], idv[:])
            id32_sb = cp.tile([128, 128], F32, tag="id32", name="id32")
            nc.gpsimd.dma_start(id32_sb[:], id32[:])

            # x in [128, 512] chunks, issued in consumption order, spread
            # over two issuing engines so neither serializes the stream.
            xk_sb = [[None] * 4 for _ in range(8)]
            xq_sb = [[None] * 2 for _ in range(8)]

            def _load_chunk(store, dram, kt, nb, cols, engine):
                t = cp.tile([128, 512], DT, tag=f"{dram.name}{kt}_{nb}",
                            name=f"{dram.name}{kt}_{nb}")
                engine.dma_start(
                    t[:], dram[kt * 128:(kt + 1) * 128, nb * 512:(nb + 1) * 512])
                store[kt][nb] = t

            for nb in range(4):
                for kt in range(8):
                    eng = nc.sync if kt % 2 == 0 else nc.gpsimd
                    _load_chunk(xk_sb, xkT, kt, nb, 512, eng)
                if nb < 2:
                    for kt in range(8):
                        eng = nc.gpsimd if kt % 2 == 0 else nc.sync
                        _load_chunk(xq_sb, xqT, kt, nb, 512, eng)

            kvT_sb = cp.tile([128, S], DT, tag="kvT", name="kvT")     # rows 0:64 K^T, 64:128 V^T
            qT_sb = cp.tile([H, QR], DT, tag="qT", name="qT")
            v_sb = cp.tile([128, NKT * (H + 1)], DT, tag="v", name="v")  # V tiles + ones col

            # ---- phase 1: projections (own PSUM scope: 6 banks, freed after) ----
            with tc.tile_pool(name="proj_ps", bufs=1, space="PSUM") as pp:
                kv_ps = [pp.tile([128, 512], F32, tag=f"kvps{nb}", name=f"kvps{nb}") for nb in range(4)]
                q_ps = [pp.tile([H, 512], F32, tag=f"qps{j}", name=f"qps{j}") for j in range(2)]
                for nb in range(4):
                    for kt in range(8):
                        nc.tensor.matmul(
                            kv_ps[nb][:],
                            wkv_sb[:, kt * 128:(kt + 1) * 128],
                            xk_sb[kt][nb][:],
                            start=(kt == 0),
                            stop=(kt == 7),
                        )
                    if nb < 2:
                        for kt in range(8):
                            nc.tensor.matmul(
                                q_ps[nb][:],
                                wq_sb[:, kt * H:(kt + 1) * H],
                                xq_sb[kt][nb][:],
                                start=(kt == 0),
                                stop=(kt == 7),
                            )
                    nc.vector.tensor_scalar(
                        kvT_sb[:, nb * 512:(nb + 1) * 512], kv_ps[nb][:],
                        bkv_sb[:], None, add)
                    if nb < 2:
                        nc.vector.tensor_scalar(
                            qT_sb[:, nb * 512:(nb + 1) * 512], q_ps[nb][:],
                            bq_sb[:], None, add)

            # ---- phase 1c + 2 PSUM pools: vtr 2 + score 2 + av 2 + otr 2 = 8 banks
            vp = tc.alloc_tile_pool(name="tr_ps", bufs=2, space="PSUM")
            sp = tc.alloc_tile_pool(name="score_ps", bufs=3, space="PSUM")
            avp = tc.alloc_tile_pool(name="av_ps", bufs=1, space="PSUM")
            op = vp

            # ---- phase 1c: V^T -> V tiles (+ ones column) ----
            nc.vector.memset(v_sb[:], 1.0)
            for kt in range(NKT):
                vt_ps = vp.tile([128, H], DT, tag="tr", name="vtr")
                nc.tensor.transpose(
                    vt_ps[:],
                    kvT_sb[64:128, kt * 128:(kt + 1) * 128],
                    idv_sb[64:64 + H, :],
                )
                nc.vector.tensor_copy(
                    v_sb[:, kt * (H + 1):kt * (H + 1) + H], vt_ps[:])

            # ---- phase 2: attention per slot ----
            for slot in range(2):
                nkt = SLOT_KT[slot]
                av = avp.tile([H + 1, 512], F32, tag=f"av{slot}", name=f"av{slot}")
                for kt in range(nkt):
                    s_ps = sp.tile([128, 512], F32, tag="score", name="score")
                    nc.tensor.matmul(
                        s_ps[:],
                        kvT_sb[0:H, kt * 128:(kt + 1) * 128],
                        qT_sb[:, slot * 512:(slot + 1) * 512],
                        start=True, stop=True,
                    )
                    w_sb = wp.tile([128, 512], DT, tag="wexp", name="wexp")
                    nc.scalar.activation(w_sb[:], s_ps[:], Exp, scale=float(SCALE))
                    if slot == 1 and kt < 8:
                        # k rows < 1024 are causally allowed for every
                        # slot-B query (q >= 1024 on all cores): no mask.
                        w_av = w_sb
                    else:
                        wm_sb = wp.tile([128, 512], DT, tag="wm", name="wm")
                        idx = slot * NKT + kt
                        nc.vector.scalar_tensor_tensor(
                            wm_sb[:], qio_sb[:], thr_sb[:, idx:idx + 1], w_sb[:],
                            ge, mult)
                        w_av = wm_sb
                    nc.tensor.matmul(
                        av[:],
                        v_sb[:, kt * (H + 1):(kt + 1) * (H + 1)],
                        w_av[:],
                        start=(kt == 0), stop=(kt == nkt - 1),
                    )
                # epilogue: transpose [65, 512] -> 4x [128, 65], normalize, store
                oav_sb = ep.tile([H + 1, 512], F32, tag="oav", name="oav")
                nc.vector.tensor_copy(oav_sb[:], av[:])
                for j in range(4):
                    tr_ps = op.tile([128, H + 1], F32, tag="tr", name="otr")
                    nc.tensor.transpose(
                        tr_ps[:],
                        oav_sb[:, j * 128:(j + 1) * 128],
                        id32_sb[0:H + 1, 0:H + 1],
                    )
                    r_sb = ep.tile([128, 1], F32, tag="recip", name="recip")
                    nc.vector.reciprocal(r_sb[:], tr_ps[:, H:H + 1])
                    o_sb = ep.tile([128, H], F32, tag="osb", name="osb")
                    nc.vector.tensor_scalar_mul(o_sb[:], tr_ps[:, 0:H], r_sb[:])
                    row = slot * CH + j * 128
                    nc.sync.dma_start(out[row:row + 128, :], o_sb[:])

            for pool in (avp, sp, vp):
                pool.release()

    nc.compile()
    return nc


def _host_inputs(x, Wq, bq, Wk, bk, Wv, bv):
    """Build the 8 per-core input maps (all SBUF-layout, fp16/f32)."""
    f16 = np.float16
    Wkv = np.concatenate([Wk, Wv], axis=1)          # [D, 128]
    wkv_np = np.zeros((128, 8 * 128), dtype=f16)
    wq_np = np.zeros((128, 8 * H), dtype=f16)
    for kt in range(8):
        wkv_np[:, kt * 128:(kt + 1) * 128] = Wkv[kt * 128:(kt + 1) * 128, :]
        wq_np[:, kt * H:(kt + 1) * H] = Wq[kt * 128:(kt + 1) * 128, :]
    bkv_np = np.concatenate([bk, bv]).reshape(128, 1).astype(np.float32)
    bq_np = bq.reshape(H, 1).astype(np.float32)
    qio_np = np.broadcast_to(
        np.arange(CH, dtype=f16), (128, CH)).copy()
    idv_np = np.concatenate([np.eye(H), np.eye(H)], axis=0).astype(f16)
    id32_np = np.eye(128, dtype=np.float32)

    in_maps = []
    for c in range(8):
        b = c // 2
        cA, cB = c % 2, 3 - c % 2
        xkT_np = np.ascontiguousarray(x[b].T).astype(f16)        # [D, S]
        xqT_np = np.concatenate(
            [x[b, cA * CH:(cA + 1) * CH].T, x[b, cB * CH:(cB + 1) * CH].T],
            axis=1).astype(f16)                                   # [D, QR]
        thr_np = np.zeros((128, 2 * NKT), dtype=f16)
        p = np.arange(128)
        for slot, ck in enumerate((cA, cB)):
            for kt in range(NKT):
                thr_np[:, slot * NKT + kt] = kt * 128 + p - ck * CH
        in_maps.append({
            "xkT": xkT_np, "xqT": xqT_np, "wkv": wkv_np, "wq": wq_np,
            "bkv": bkv_np, "bq": bq_np, "qio": qio_np, "thr": thr_np,
            "idv": idv_np, "id32": id32_np,
        })
    return in_maps


def _gather(results, dtype):
    y = np.zeros((B, S, H), dtype=dtype)
    for c in range(8):
        b = c // 2
        cA, cB = c % 2, 3 - c % 2
        o = results[c]["out"]
        y[b, cA * CH:(cA + 1) * CH] = o[:CH]
        y[b, cB * CH:(cB + 1) * CH] = o[CH:]
    return y


def get_nc():
    if "nc" not in _CACHE:
        _CACHE["nc"] = _build_nc()
    return _CACHE["nc"]


def kernel(x, Wq, bq, Wk, bk, Wv, bv, _trace=False, _trace_kwargs=None):
    from concourse.bass_utils import run_bass_kernel_spmd

    x = np.asarray(x, dtype=np.float32)
    Wq, bq = np.asarray(Wq, np.float32), np.asarray(bq, np.float32)
    Wk, bk = np.asarray(Wk, np.float32), np.asarray(bk, np.float32)
    Wv, bv = np.asarray(Wv, np.float32), np.asarray(bv, np.float32)

    nc = get_nc()
    in_maps = _host_inputs(x, Wq, bq, Wk, bk, Wv, bv)
    res = run_bass_kernel_spmd(
        nc, in_maps, core_ids=list(range(8)),
        trace=_trace, **(_trace_kwargs or {}))
    _CACHE["last_result"] = res
    return _gather(res.results, x.dtype)
